# revision 29
# baseline (speedup 1.0000x reference)
"""GAT neighbor-aggregation kernel for Trainium2, 8-core data-parallel.

Math (per batch b):
  vu = ea @ U2 ; iv = ea @ W2
  logits[i,j] = sum_c yita_c * leaky_relu(vu[i,c] + iv[j,c], 0.2)
  alpha = softmax_j(where(adj>0, logits, -1e12))
  out = leaky_relu(alpha @ ea, 0.2)

Device decomposition (quantized-interpolation table matmul):
  leaky_relu(v) = 0.8*relu(v) + 0.2*v, so with s = vu*|yita|, t = iv*|yita|,
  w_c = 0.8*sign(yita_c):
    logits[i,j] = 0.2*p_i + 0.2*q_j + sum_c w_c * relu(s[i,c] + t[j,c])
  (p_i dropped: softmax row constant).  relu(s+t) is piecewise linear in s,
  so with per-(core,c) uniform levels sigma_{c,0..L-1} over [min_i s, max_i s]:
    relu(s_ic + t_jc) = (1-u)*relu(sigma_l + t_jc) + u*relu(sigma_{l+1} + t_jc)
  EXACTLY unless the cell straddles the kink -t_jc (error <= cell/4 there;
  measured end-to-end rel error ~9e-3 at L=16, budget 2e-2).  Therefore
    R[i,j] ~= sum_{(c,l)} W[(c,l),i] * T[(c,l),j]
  one dense matmul with contraction K = 64*L = 1024 (8 chained 128-row
  matmuls per 128-i output tile, ~215ns each).  W (interpolation weights,
  2 nonzeros per c per column) and T (tables relu(sigma+t)) are
  host-precomputed from the O(e*c) quantities; the O(e^2) work runs on the
  tensor engine at full 128-wide utilization instead of elementwise-bound.
  Uploads are split across the three DMA queues and chunk-pipelined so the
  PE starts as soon as the first W/T pieces land.

  The adjacency mask (+ the 0.2*q_j column bias) is one more accumulated
  matmul adding -60000*(1-adj)+0.2*q_j (identity lhsT, fp16 rhs), so
  exp(masked logit) flushes to 0.  Softmax runs without max-subtraction
  (|logits| < 8, fp16-exp safe): exp in 256-col chunks (scalar engine,
  PSUM->SBUF fp16), transpose per 128 cols (PE), copy (vector engine),
  alpha @ eaS matmul (PE, ones column = denominator), then
  out = Prelu(P * 1/denom, 0.2) in fp16.

Sharding: core = 2*b + h handles batch b, query rows i in [256h, 256h+256).
"""

import numpy as np
from contextlib import ExitStack

import concourse.bass as bass
import concourse.tile as tile
from concourse import bacc, mybir
from concourse.bass_utils import run_bass_kernel_spmd

F32 = mybir.dt.float32
F16 = mybir.dt.float16
OP = mybir.AluOpType

BSZ, E, C = 4, 512, 64
NCORE = 8
IPC = E // 2          # 256 query rows per core
NTILE = IPC // 128    # 2 logits tiles of 128 i-rows
LVL = 16              # interpolation levels per channel
K = C * LVL           # table contraction size (1024)
NCH = K // 128        # 8 contraction chunks of 128
N_WARM = 56           # PE warmup matmuls issued while input DMAs are in flight
MASKV = -60000.0      # mask add value; exp(-60000) == 0 in fp16/fp32


def _build_program():
    nc = bacc.Bacc(
        "TRN2",
        target_bir_lowering=False,
        debug=False,
        enable_asserts=False,
        num_devices=NCORE,
    )
    # T in 4 parts of 2 chunks, W0 in 2 parts of 4 chunks: the PE starts on
    # (W0a, tab0) while the rest is on the wire
    tab_aps = [
        nc.dram_tensor(f"tab{i}", [128, 2 * E], F16, kind="ExternalInput").ap()
        for i in range(NCH // 2)
    ]
    # W interleaved by chunk: w{i}[p, (q t m)] holds chunks 4i..4i+4 for BOTH
    # output tiles, so the q-interleaved matmul stream needs only the part
    # that has landed
    w_aps = [
        nc.dram_tensor(f"wi{i}", [128, 4 * NTILE * 128], F16, kind="ExternalInput").ap()
        for i in range(2)
    ]
    wident_ap = nc.dram_tensor("wident", [128, 128], F16, kind="ExternalInput").ap()
    eaS_ap = nc.dram_tensor("eaS", [128, 4 * (C + 1)], F16, kind="ExternalInput").ap()
    madj_aps = [
        nc.dram_tensor(f"madj{t}", [128, E], F16, kind="ExternalInput").ap()
        for t in range(NTILE)
    ]
    out_ap = nc.dram_tensor("out", [IPC, C], F16, kind="ExternalOutput").ap()

    with tile.TileContext(nc) as tc:
        with ExitStack() as ctx:
            singles = ctx.enter_context(tc.tile_pool(name="singles", bufs=1))
            ps_logits = ctx.enter_context(
                tc.tile_pool(name="ps_logits", bufs=2, space="PSUM")
            )
            ps_tp = ctx.enter_context(tc.tile_pool(name="ps_tp", bufs=2, space="PSUM"))
            ps_fm = ctx.enter_context(tc.tile_pool(name="ps_fm", bufs=2, space="PSUM"))
            small = ctx.enter_context(tc.tile_pool(name="small", bufs=6))
            epool = ctx.enter_context(tc.tile_pool(name="epool", bufs=4))
            atpool = ctx.enter_context(tc.tile_pool(name="atpool", bufs=4))

            # ---- PE warmup: no input deps, runs during the DMA fill ----
            warm_sb = singles.tile([128, C], F16, tag="warm")
            nc.vector.memset(warm_sb[:], 0.0)
            warm_ps = ps_fm.tile([C, C], F32, tag="fm")
            for _ in range(N_WARM):
                nc.tensor.matmul(warm_ps[:], lhsT=warm_sb[:, 0:C], rhs=warm_sb[:])

            # ---- input DMAs, balanced across the three DMA queues; the
            # pieces gating the first table matmuls (W0a, tab0) go first ----
            # queue plan (just-in-time arrival order for the PE stream):
            #   scalar: wiA, wiB | sync: tab0, tab2, madj0, ident
            #   gpsimd: tab1, tab3, madj1, eaS
            wi = []
            for i in range(2):
                wt = singles.tile([128, 4, NTILE, 128], F16, tag=f"wi{i}")
                nc.scalar.dma_start(
                    wt[:], w_aps[i].rearrange("p (q t m) -> p q t m", q=4, t=NTILE)
                )
                wi.append(wt)
            tabs = [
                singles.tile([128, 2, E], F16, tag=f"tab{i}", name=f"tab{i}")
                for i in range(NCH // 2)
            ]
            nc.sync.dma_start(tabs[0][:], tab_aps[0].rearrange("p (q j) -> p q j", q=2))
            nc.gpsimd.dma_start(tabs[1][:], tab_aps[1].rearrange("p (q j) -> p q j", q=2))
            nc.sync.dma_start(tabs[2][:], tab_aps[2].rearrange("p (q j) -> p q j", q=2))
            nc.gpsimd.dma_start(tabs[3][:], tab_aps[3].rearrange("p (q j) -> p q j", q=2))
            madj = []
            for t in range(NTILE):
                mt = singles.tile([128, E], F16, tag=f"madj{t}", name=f"madj{t}")
                (nc.sync if t == 0 else nc.gpsimd).dma_start(mt[:], madj_aps[t][:])
                madj.append(mt)
            ident_sb = singles.tile([128, 128], F16, tag="ident")
            nc.sync.dma_start(ident_sb[:], wident_ap[:])
            eaS = singles.tile([128, 4, C + 1], F16, tag="eaS")
            nc.gpsimd.dma_start(eaS[:], eaS_ap.rearrange("p (ch c) -> p ch c", ch=4))

            # ---- logits: q-interleaved dense matmuls, both tiles per chunk
            # arrival; masks close each tile's accumulation ----
            logits = [
                ps_logits.tile([128, E], F32, tag="logits", name=f"logits{t}")
                for t in range(NTILE)
            ]
            for q in range(NCH):
                for t in range(NTILE):
                    nc.tensor.matmul(
                        logits[t][:],
                        lhsT=wi[q // 4][:, q % 4, t, :],
                        rhs=tabs[q // 2][:, q % 2, :],
                        start=(q == 0),
                        stop=False,
                        skip_group_check=True,
                    )
            for t in range(NTILE):
                # mask + column bias: logits += -60000*(1-adj) + 0.2*q_j
                nc.tensor.matmul(
                    logits[t][:],
                    lhsT=ident_sb[:],
                    rhs=madj[t][:],
                    start=False,
                    stop=True,
                    skip_group_check=True,
                )
            for t in range(NTILE):
                logits_ps = logits[t]
                # softmax numerator (no max-sub): exp per 256-col chunk, then
                # transpose/copy/fm per 128-col chunk; the ones column of eaS
                # yields the denominator through the fm matmul
                fm_ps = ps_fm.tile([128, C + 1], F32, tag="fm")
                for hh in range(2):
                    e_h = epool.tile([128, 256], F16, tag="esb")
                    nc.scalar.activation(
                        e_h[:], logits_ps[:, hh * 256 : (hh + 1) * 256],
                        mybir.ActivationFunctionType.Exp, bias=0.0, scale=1.0,
                    )
                    for cc in range(2):
                        ch = hh * 2 + cc
                        aT = atpool.tile([128, 128], F16, tag="aT")
                        tp = ps_tp.tile([128, 128], F16, tag="tp")
                        nc.tensor.transpose(
                            tp[:], e_h[:, cc * 128 : (cc + 1) * 128], ident_sb
                        )
                        nc.vector.tensor_copy(aT[:], tp[:])
                        nc.tensor.matmul(
                            fm_ps[:],
                            lhsT=aT[:],
                            rhs=eaS[:, ch, :],
                            start=(ch == 0),
                            stop=(ch == 3),
                        )
                # out = leaky_relu(P / denom); rec > 0 so
                # out = max(P*rec, 0.2*(P*rec)), all on the vector engine
                rec = small.tile([128, 1], F32, tag="rec")
                nc.vector.reciprocal(rec[:], fm_ps[:, C : C + 1])
                od = small.tile([128, C], F32, tag="od")
                nc.vector.tensor_scalar(od[:], fm_ps[:, 0:C], rec[:], None, OP.mult)
                out_sb = small.tile([128, C], F16, tag="outsb")
                nc.vector.scalar_tensor_tensor(
                    out_sb[:], od[:], 0.2, od[:], OP.mult, OP.max
                )
                nc.sync.dma_start(out_ap[t * 128 : (t + 1) * 128, :], out_sb[:])

    nc.finalize()
    return nc


_NC = None


def _get_nc():
    global _NC
    if _NC is None:
        _NC = _build_program()
    return _NC


def _host_prep(edge_attr, edge_adj, W_2, U_2, yita):
    edge_attr = np.asarray(edge_attr, dtype=np.float32)
    edge_adj = np.asarray(edge_adj)
    W_2 = np.asarray(W_2, dtype=np.float32)
    U_2 = np.asarray(U_2, dtype=np.float32)
    yita = np.asarray(yita, dtype=np.float32)

    y = yita[:, 0]
    ay = np.abs(y)
    w = (0.8 * np.sign(y)).astype(np.float32)
    wident = np.eye(128, dtype=np.float16)

    in_maps = []
    for core in range(NCORE):
        b, h = divmod(core, 2)
        ea = edge_attr[b]                      # [E, C]
        vu = ea @ U_2
        iv = ea @ W_2
        s = vu * ay[None, :]                   # [E, C]
        t = iv * ay[None, :]                   # [E, C]
        q = iv @ y                             # [E]

        sh = s[h * IPC : (h + 1) * IPC]        # [IPC, C]
        lo, hi = sh.min(0), sh.max(0)          # [C]
        span = np.maximum(hi - lo, 1e-6)
        sig = lo[None, :] + span[None, :] * (
            np.arange(LVL, dtype=np.float32)[:, None] / (LVL - 1)
        )
        # tables T[(c,l), j] = relu(sig[l,c] + t[j,c]), contraction-major
        T = np.maximum(sig.T[:, :, None] + t.T[:, None, :], 0.0)  # [C, LVL, E]
        T = T.reshape(K, E).astype(np.float16)

        # interpolation weights W[(c,l), i_local]
        delta = span / (LVL - 1)
        u = (sh - lo[None, :]) / delta[None, :]
        l0 = np.clip(np.floor(u).astype(np.int64), 0, LVL - 2)
        frac = (u - l0).astype(np.float32)
        Wt = np.zeros((C, LVL, IPC), dtype=np.float32)
        ii = np.arange(IPC)
        for c in range(C):
            Wt[c, l0[:, c], ii] += w[c] * (1.0 - frac[:, c])
            Wt[c, l0[:, c] + 1, ii] += w[c] * frac[:, c]
        Wt = Wt.reshape(K, IPC).astype(np.float16)

        # eaS[:, ch, 0:C] = ea in chunk layout, col C = 1 (denominator)
        eaS = np.empty((128, 4, C + 1), dtype=np.float16)
        for chn in range(4):
            rows = slice(chn * 128, (chn + 1) * 128)
            eaS[:, chn, 0:C] = ea[rows].astype(np.float16)
            eaS[:, chn, C] = np.float16(1.0)

        # madj{t}[r, j] = (MASKV if adj==0 else 0) + 0.2*q_j for i = base + r
        adjh = edge_adj[b, h * IPC : (h + 1) * IPC, :]
        madj = np.where(adjh > 0, 0.0, MASKV) + 0.2 * q[None, :]
        madj = madj.astype(np.float16).reshape(NTILE, 128, E)

        im = {
            "wident": wident,
            "eaS": np.ascontiguousarray(eaS.reshape(128, 4 * (C + 1))),
        }
        for t in range(NTILE):
            im[f"madj{t}"] = np.ascontiguousarray(madj[t])
        Tq = T.reshape(NCH, 128, E)
        for i in range(NCH // 2):
            im[f"tab{i}"] = np.ascontiguousarray(
                Tq[2 * i : 2 * i + 2].transpose(1, 0, 2).reshape(128, 2 * E)
            )
        # wi{i}[p, (q t m)] = Wt[(4i+q)*128+p, t*128+m]
        Wq = Wt.reshape(NCH, 128, NTILE, 128)
        for i in range(2):
            im[f"wi{i}"] = np.ascontiguousarray(
                Wq[4 * i : 4 * i + 4].transpose(1, 0, 2, 3).reshape(
                    128, 4 * NTILE * 128
                )
            )
        in_maps.append(im)
    return in_maps


def kernel(edge_attr, edge_adj, e_max=None, mask=None, W_2=None, U_2=None, yita=None):
    nc = _get_nc()
    in_maps = _host_prep(edge_attr, edge_adj, W_2, U_2, yita)
    res = run_bass_kernel_spmd(nc, in_maps, core_ids=list(range(NCORE)))
    out = np.empty((BSZ, E, C), dtype=np.float32)
    for core in range(NCORE):
        b, h = divmod(core, 2)
        out[b, h * IPC : (h + 1) * IPC, :] = res.results[core]["out"].astype(
            np.float32
        )
    return out
